# revision 1
# baseline (speedup 1.0000x reference)
"""Trainium2 Bass kernel for nn_ConditionalMLN.

Math: the reference reduces exactly (cart.sum(-1) == 1 algebraically) to
    out = sum_r w_r * (G + cnt_r - S_r),   S_r = sum_g flag[r,g] * Z[r,g]
    Z = prod_k t_k,  t_k = select(mask_k, p[i_k], 1 - p[i_k])

Device strategy (R sharded over 8 cores, 2 rules each -> 1.2M table
lookups per core):
  * host builds a doubled bf16 table: table2[2i+m] = m ? p_i : 1-p_i,
    plus one zero row; flag==0 elements point plane 0 at the zero row
    (dead -> Z=0), folding mask and flag away entirely.
  * dma_gather (SWDGE row gather, 256B rows of 128 bf16) fetches the
    row containing each element's entry: row = idx2>>7 (int16-safe),
    one descriptor per element, 8192 elements per instruction.
  * DVE extracts entry e = idx2&127 from each row via
    onehot(iota==e) multiply + windowed tensor_reduce(axis=X), then
    multiplies the three planes and segment-reduces per rule.
  * host: S_r = acc.sum(axis=0); out = sum_r w_r (G + cnt_r - S_r).
"""

import numpy as np
import ml_dtypes

R, G, K, N = 16, 200000, 3, 2000000
NCORES = 8
P = 128
RLOC = R // NCORES              # 2 rules per core
GCOLS = (G + P - 1) // P        # 1563 columns per rule (G padded to 200064)
GPAD = GCOLS * P                # 200064 slots per rule
COLS = RLOC * GCOLS             # 3126 columns per core
NSLOT = COLS * P                # 400128 slots per core
NROWS = 2 * N // P              # 31250 rows of 128 bf16 entries in table2
DEAD = 2 * N                    # flat index of the zero row (row NROWS, e=0)
TCOLS = 64                      # output columns per dma_gather tile
TSLOT = TCOLS * P               # 8192 slots per tile
NTILE = (COLS + TCOLS - 1) // TCOLS   # 49 tiles (last tile 54 cols)

_CACHE = {}


def _build_program():
    from concourse import bass, mybir, library_config

    nc = bass.Bass("TRN2", target_bir_lowering=False, debug=False,
                   num_devices=NCORES)

    f32, bf16, i16 = mybir.dt.float32, mybir.dt.bfloat16, mybir.dt.int16

    tbl_d = nc.declare_dram_parameter("tbl", [NROWS + 1, P], bf16,
                                      isOutput=False)
    idx_d = [nc.declare_dram_parameter(f"idx{k}", [P, NSLOT // 16], i16,
                                       isOutput=False) for k in range(K)]
    e_d = [nc.declare_dram_parameter(f"e{k}", [P, COLS], bf16,
                                     isOutput=False) for k in range(K)]
    iota_d = nc.declare_dram_parameter("iota", [P, P], bf16, isOutput=False)
    y_d = nc.declare_dram_parameter("y", [P, RLOC], f32, isOutput=True)

    idx_s = [[nc.alloc_sbuf_tensor(f"idx{k}_{b}", [P, TCOLS * 8], i16)
              for b in range(2)] for k in range(K)]
    rows_s = [[nc.alloc_sbuf_tensor(f"rows{k}_{b}", [P, TSLOT], bf16)
               for b in range(2)] for k in range(K)]
    e_s = [nc.alloc_sbuf_tensor(f"e{k}_s", [P, COLS], bf16) for k in range(K)]
    iota_s = nc.alloc_sbuf_tensor("iota_s", [P, P], bf16)
    oh_s = nc.alloc_sbuf_tensor("oh_s", [P, TSLOT], bf16)
    ext_s = [nc.alloc_sbuf_tensor(f"ext{k}_s", [P, COLS], f32)
             for k in range(K)]
    acc_s = nc.alloc_sbuf_tensor("acc_s", [P, RLOC], f32)

    AluOp = mybir.AluOpType
    X = mybir.AxisListType.X

    def tile_cols(t):
        return min(TCOLS, COLS - t * TCOLS)

    with (
        nc.Block() as block,
        nc.semaphore("dsem") as dsem,     # e planes + iota loaded
        nc.semaphore("isem") as isem,     # idx tile loads
        nc.semaphore("gsem") as gsem,     # gather completions
        nc.semaphore("vsem") as vsem,     # vector tile completions
        nc.semaphore("fsem") as fsem,     # final vector done
        nc.semaphore("osem") as osem,
    ):
        @block.sync
        def _(sync):
            for k in range(K):
                sync.dma_start(out=e_s[k].ap(), in_=e_d[k][:]).then_inc(dsem, 16)
            sync.dma_start(out=iota_s.ap(), in_=iota_d[:]).then_inc(dsem, 16)
            for t in range(NTILE):
                c = tile_cols(t) * 8
                if t >= 2:
                    # idx buf t%2 free once gathers of tile t-2 completed
                    sync.wait_ge(gsem, 16 * K * (t - 1))
                for k in range(K):
                    sync.dma_start(
                        out=idx_s[k][t % 2].ap()[:, :c],
                        in_=idx_d[k][:, t * TCOLS * 8: t * TCOLS * 8 + c],
                    ).then_inc(isem, 16)
            sync.wait_ge(fsem, 1)
            sync.dma_start(out=y_d[:], in_=acc_s.ap()).then_inc(osem, 16)
            sync.wait_ge(osem, 16)

        @block.gpsimd
        def _(g):
            g.load_library(library_config.mlp)
            ns_regs = {}
            for t in range(NTILE):
                ns_t = tile_cols(t) * P
                if ns_t not in ns_regs:
                    ns_regs[ns_t] = g.to_reg(ns_t)
            for t in range(NTILE):
                nc_t = tile_cols(t)
                ns = nc_t * P
                g.wait_ge(isem, 16 * K * (t + 1))
                if t >= 2:
                    g.wait_ge(vsem, t - 1)   # rows buf free
                for k in range(K):
                    out3 = rows_s[k][t % 2].ap()[:, :ns].rearrange(
                        "p (c l) -> p c l", l=P)
                    g.dma_gather(
                        out_ap=out3,
                        in_ap=tbl_d[:],
                        idxs_ap=idx_s[k][t % 2].ap()[:, :nc_t * 8],
                        num_idxs=ns,
                        num_idxs_reg=ns_regs[ns],
                        elem_size=P,
                        single_packet=False,
                    ).then_inc(gsem, 16)

        @block.vector
        def _(v):
            v.wait_ge(dsem, 16 * (K + 1))
            for t in range(NTILE):
                nc_t = tile_cols(t)
                ns = nc_t * P
                c0 = t * TCOLS
                v.wait_ge(gsem, 16 * K * (t + 1))
                iota_b = iota_s.ap().unsqueeze(1).broadcast_to([P, nc_t, P])
                oh3 = oh_s.ap()[:, :ns].rearrange("p (c l) -> p c l", l=P)
                last = None
                for k in range(K):
                    e_b = e_s[k].ap()[:, c0:c0 + nc_t].unsqueeze(2) \
                        .broadcast_to([P, nc_t, P])
                    rows3 = rows_s[k][t % 2].ap()[:, :ns].rearrange(
                        "p (c l) -> p c l", l=P)
                    v.tensor_tensor(out=oh3, in0=iota_b, in1=e_b,
                                    op=AluOp.is_equal)
                    v.tensor_tensor(out=rows3, in0=rows3, in1=oh3,
                                    op=AluOp.mult)
                    last = v.tensor_reduce(
                        ext_s[k].ap()[:, c0:c0 + nc_t], rows3, X, AluOp.add)
                last.then_inc(vsem, 1)
            # Z = ext0*ext1*ext2 (into ext0), then per-rule segment reduce
            z = ext_s[0].ap()
            v.tensor_tensor(out=z, in0=z, in1=ext_s[1].ap(), op=AluOp.mult)
            v.tensor_tensor(out=z, in0=z, in1=ext_s[2].ap(), op=AluOp.mult)
            red = None
            for r in range(RLOC):
                red = v.tensor_reduce(
                    acc_s.ap()[:, r:r + 1],
                    z[:, r * GCOLS:(r + 1) * GCOLS], X, AluOp.add)
            red.then_inc(fsem, 1)

    from concourse.library_overlay import lower_extended_insts
    lower_extended_insts(nc)
    return nc


def _prep_core(idx2):
    """idx2: [RLOC, G] int64 flat table2 indices (mask/flag folded).

    Returns (idx16_wrapped [P, NSLOT//16] int16, e_plane [P, COLS] bf16).
    Slot j = r*GPAD + g; within-core layout: partition j%128... see below.
    """
    pad = np.full((RLOC, GPAD - G), DEAD, dtype=np.int64)
    flat = np.concatenate([idx2, pad], axis=1).reshape(NSLOT)  # slot-ordered
    row = (flat >> 7).astype(np.int16)
    e = (flat & 127).astype(np.float32).astype(ml_dtypes.bfloat16)

    # e-plane: slot j -> [partition j%128, col j//128]
    e_plane = np.ascontiguousarray(e.reshape(COLS, P).T)

    # idx16: per gather tile t (TSLOT slots), wrapped-16 layout replicated
    # across the 8 q7 core groups: within tile, slot s -> [part s%16 (+16m),
    # col s//16]; tiles concatenated along columns.
    w = row.reshape(NSLOT // 16, 16).T                  # [16, NSLOT//16]
    idx16 = np.ascontiguousarray(np.tile(w, (8, 1)))    # [128, NSLOT//16]
    return idx16, e_plane


def prepare_in_maps(posterior_prob, latent_var_inds, latent_neg_mask,
                    obs_zero_flag):
    # table2[2i]=1-p_i, table2[2i+1]=p_i, plus a zero row at the end
    p = np.asarray(posterior_prob).astype(np.float32)
    t2 = np.empty((NROWS + 1) * P, dtype=ml_dtypes.bfloat16)
    t2[0:2 * N:2] = (1.0 - p).astype(ml_dtypes.bfloat16)
    t2[1:2 * N:2] = p.astype(ml_dtypes.bfloat16)
    t2[2 * N:] = ml_dtypes.bfloat16(0.0)
    tbl = t2.reshape(NROWS + 1, P)

    iota = np.tile(np.arange(P, dtype=np.float32), (P, 1)) \
        .astype(ml_dtypes.bfloat16)

    inds = np.asarray(latent_var_inds).astype(np.int64)
    mask = np.asarray(latent_neg_mask).astype(np.int64)
    idx2_all = 2 * inds + mask                      # [R, G, K]
    # fold flag into plane 0: flag==False -> dead (zero row) -> Z=0
    dead0 = np.where(np.asarray(obs_zero_flag), idx2_all[:, :, 0], DEAD)
    idx2_all[:, :, 0] = dead0

    in_maps = []
    for c in range(NCORES):
        rules = slice(RLOC * c, RLOC * (c + 1))
        m = {"tbl": tbl, "iota": iota}
        for k in range(K):
            idx16, e_plane = _prep_core(idx2_all[rules, :, k])
            m[f"idx{k}"] = idx16
            m[f"e{k}"] = e_plane
        in_maps.append(m)
    return in_maps


def kernel(posterior_prob, observed_rule_cnts, rule_weights,
           latent_var_inds, latent_neg_mask, obs_zero_flag):
    observed_rule_cnts = np.asarray(observed_rule_cnts)
    rule_weights = np.asarray(rule_weights)

    if "nc" not in _CACHE:
        _CACHE["nc"] = _build_program()
    nc = _CACHE["nc"]

    in_maps = prepare_in_maps(posterior_prob, latent_var_inds,
                              latent_neg_mask, obs_zero_flag)

    from concourse.bass_utils import run_bass_kernel_spmd
    res = run_bass_kernel_spmd(nc, in_maps, core_ids=list(range(NCORES)))

    s = np.empty(R, dtype=np.float64)
    for c in range(NCORES):
        part = res.results[c]["y"].astype(np.float64).sum(axis=0)   # [RLOC]
        s[RLOC * c:RLOC * (c + 1)] = part
    scores = np.float64(G) + observed_rule_cnts.astype(np.float64) - s
    out = rule_weights.astype(np.float64) @ scores
    return np.asarray([out], dtype=np.float32)



# revision 6
# speedup vs baseline: 2.0685x; 2.0685x over previous
"""Trainium2 Bass kernel for nn_ConditionalMLN.

Math: the reference reduces exactly (cart.sum(-1) == 1 algebraically) to
    out = sum_r w_r * (G + cnt_r - S_r),   S_r = sum_g flag[r,g] * Z[r,g]
    Z = prod_k t_k,  t_k = select(mask_k, p[i_k], 1 - p[i_k])

Device strategy (R sharded over 8 cores, 2 rules each -> 1.2M table
lookups per core):
  * host builds a doubled bf16 table: table2[2i+m] = m ? p_i : 1-p_i,
    plus one zero row; flag==0 elements point plane 0 at the zero row
    (dead -> Z=0), folding mask and flag away entirely.
  * dma_gather (SWDGE row gather, 256B rows of 128 bf16) fetches the
    row containing each element's entry: row = idx2>>7 (int16-safe),
    one descriptor per element, 8192 elements per instruction.
  * DVE extracts entry e = idx2&127 from each row via
    onehot(iota==e) multiply + windowed tensor_reduce(axis=X), then
    multiplies the three planes and segment-reduces per rule.
  * host: S_r = acc.sum(axis=0); out = sum_r w_r (G + cnt_r - S_r).
"""

import numpy as np
import ml_dtypes

R, G, K, N = 16, 200000, 3, 2000000
NCORES = 8
P = 128
RLOC = R // NCORES              # 2 rules per core
GCOLS = (G + P - 1) // P        # 1563 columns per rule (G padded to 200064)
GPAD = GCOLS * P                # 200064 slots per rule
COLS = RLOC * GCOLS             # 3126 columns per core
NSLOT = COLS * P                # 400128 slots per core
NROWS = 2 * N // P              # 31250 rows of 128 bf16 entries in table2
DEAD = 2 * N                    # flat index of the zero row (row NROWS, e=0)
TCOLS = 32                      # output columns per dma_gather tile
TSLOT = TCOLS * P               # 4096 slots per tile
NTILE = (COLS + TCOLS - 1) // TCOLS   # 98 tiles (last tile 22 cols)

_CACHE = {}


def _build_program():
    from concourse import bass, mybir, library_config

    nc = bass.Bass("TRN2", target_bir_lowering=False, debug=False,
                   num_devices=NCORES, num_swdge_queues=4)

    f32, bf16, i16 = mybir.dt.float32, mybir.dt.bfloat16, mybir.dt.int16

    tbl_d = nc.declare_dram_parameter("tbl", [NROWS + 1, P], bf16,
                                      isOutput=False)
    idx_d = [nc.declare_dram_parameter(f"idx{k}", [P, NSLOT // 16], i16,
                                       isOutput=False) for k in range(K)]
    e_d = [nc.declare_dram_parameter(f"e{k}", [P, COLS], bf16,
                                     isOutput=False) for k in range(K)]
    iota_d = nc.declare_dram_parameter("iota", [P, P], bf16, isOutput=False)
    y_d = nc.declare_dram_parameter("y", [P, RLOC], f32, isOutput=True)

    NQ = 4
    idx_s = [[nc.alloc_sbuf_tensor(f"idx{k}_{b}", [P, TCOLS * 8], i16)
              for b in range(NQ)] for k in range(K)]
    rows_s = [[nc.alloc_sbuf_tensor(f"rows{k}_{b}", [P, TSLOT], bf16)
               for b in range(NQ)] for k in range(K)]
    e_s = [nc.alloc_sbuf_tensor(f"e{k}_s", [P, COLS], bf16) for k in range(K)]
    iota_s = nc.alloc_sbuf_tensor("iota_s", [P, P], bf16)
    oh_s = nc.alloc_sbuf_tensor("oh_s", [P, TSLOT], bf16)
    ext_s = [nc.alloc_sbuf_tensor(f"ext{k}_s", [P, COLS], f32)
             for k in range(K)]
    acc_s = nc.alloc_sbuf_tensor("acc_s", [P, RLOC], f32)

    AluOp = mybir.AluOpType
    X = mybir.AxisListType.X

    def tile_cols(t):
        return min(TCOLS, COLS - t * TCOLS)

    with (
        nc.Block() as block,
        nc.semaphore("dsem") as dsem,     # e planes + iota loaded
        nc.semaphore("isem") as isem,     # idx tile loads
        nc.semaphore("gsem0") as gsem0,   # gather completions, queue 0
        nc.semaphore("gsem1") as gsem1,
        nc.semaphore("gsem2") as gsem2,
        nc.semaphore("gsem3") as gsem3,
        nc.semaphore("vsem") as vsem,     # vector tile completions
        nc.semaphore("fsem") as fsem,     # final vector done
        nc.semaphore("osem") as osem,
    ):
        gsems = [gsem0, gsem1, gsem2, gsem3]

        @block.sync
        def _(sync):
            for k in range(K):
                sync.dma_start(out=e_s[k].ap(), in_=e_d[k][:]).then_inc(dsem, 16)
            sync.dma_start(out=iota_s.ap(), in_=iota_d[:]).then_inc(dsem, 16)
            for t in range(NTILE):
                c = tile_cols(t) * 8
                if t >= NQ:
                    # idx buf t%NQ free once gathers of tile t-NQ completed
                    sync.wait_ge(gsems[t % NQ], 16 * K * (t // NQ))
                for k in range(K):
                    sync.dma_start(
                        out=idx_s[k][t % NQ].ap()[:, :c],
                        in_=idx_d[k][:, t * TCOLS * 8: t * TCOLS * 8 + c],
                    ).then_inc(isem, 16)
            sync.wait_ge(fsem, 1)
            sync.dma_start(out=y_d[:], in_=acc_s.ap()).then_inc(osem, 16)
            sync.wait_ge(osem, 16)

        @block.gpsimd
        def _(g):
            g.load_library(library_config.mlp)
            ns_regs = {}
            for t in range(NTILE):
                ns_t = tile_cols(t) * P
                if ns_t not in ns_regs:
                    ns_regs[ns_t] = g.to_reg(ns_t)
            for t in range(NTILE):
                nc_t = tile_cols(t)
                ns = nc_t * P
                g.wait_ge(isem, 16 * K * (t + 1))
                if t >= NQ:
                    g.wait_ge(vsem, t - NQ + 1)   # rows buf free
                for k in range(K):
                    out3 = rows_s[k][t % NQ].ap()[:, :ns].rearrange(
                        "p (c l) -> p c l", l=P)
                    g.dma_gather(
                        out_ap=out3,
                        in_ap=tbl_d[:],
                        idxs_ap=idx_s[k][t % NQ].ap()[:, :nc_t * 8],
                        num_idxs=ns,
                        num_idxs_reg=ns_regs[ns],
                        elem_size=P,
                        single_packet=False,
                        queue_num=t % NQ,
                    ).then_inc(gsems[t % NQ], 16)

        @block.vector
        def _(v):
            v.wait_ge(dsem, 16 * (K + 1))
            for t in range(NTILE):
                nc_t = tile_cols(t)
                ns = nc_t * P
                c0 = t * TCOLS
                v.wait_ge(gsems[t % NQ], 16 * K * (t // NQ + 1))
                iota_b = iota_s.ap().unsqueeze(1).broadcast_to([P, nc_t, P])
                oh3 = oh_s.ap()[:, :ns].rearrange("p (c l) -> p c l", l=P)
                last = None
                for k in range(K):
                    e_b = e_s[k].ap()[:, c0:c0 + nc_t].unsqueeze(2) \
                        .broadcast_to([P, nc_t, P])
                    rows3 = rows_s[k][t % NQ].ap()[:, :ns].rearrange(
                        "p (c l) -> p c l", l=P)
                    v.tensor_tensor(out=oh3, in0=iota_b, in1=e_b,
                                    op=AluOp.is_equal)
                    v.tensor_tensor(out=rows3, in0=rows3, in1=oh3,
                                    op=AluOp.mult)
                    last = v.tensor_reduce(
                        ext_s[k].ap()[:, c0:c0 + nc_t], rows3, X, AluOp.add)
                last.then_inc(vsem, 1)
            # Z = ext0*ext1*ext2 (into ext0), then per-rule segment reduce
            z = ext_s[0].ap()
            v.tensor_tensor(out=z, in0=z, in1=ext_s[1].ap(), op=AluOp.mult)
            v.tensor_tensor(out=z, in0=z, in1=ext_s[2].ap(), op=AluOp.mult)
            red = None
            for r in range(RLOC):
                red = v.tensor_reduce(
                    acc_s.ap()[:, r:r + 1],
                    z[:, r * GCOLS:(r + 1) * GCOLS], X, AluOp.add)
            red.then_inc(fsem, 1)

    from concourse.library_overlay import lower_extended_insts
    lower_extended_insts(nc)
    return nc


def _prep_core(idx2):
    """idx2: [RLOC, G] int64 flat table2 indices (mask/flag folded).

    Returns (idx16_wrapped [P, NSLOT//16] int16, e_plane [P, COLS] bf16).
    Slot j = r*GPAD + g; within-core layout: partition j%128... see below.
    """
    pad = np.full((RLOC, GPAD - G), DEAD, dtype=np.int64)
    flat = np.concatenate([idx2, pad], axis=1).reshape(NSLOT)  # slot-ordered
    row = (flat >> 7).astype(np.int16)
    e = (flat & 127).astype(np.float32).astype(ml_dtypes.bfloat16)

    # e-plane: slot j -> [partition j%128, col j//128]
    e_plane = np.ascontiguousarray(e.reshape(COLS, P).T)

    # idx16: per gather tile t (TSLOT slots), wrapped-16 layout replicated
    # across the 8 q7 core groups: within tile, slot s -> [part s%16 (+16m),
    # col s//16]; tiles concatenated along columns.
    w = row.reshape(NSLOT // 16, 16).T                  # [16, NSLOT//16]
    idx16 = np.ascontiguousarray(np.tile(w, (8, 1)))    # [128, NSLOT//16]
    return idx16, e_plane


def prepare_in_maps(posterior_prob, latent_var_inds, latent_neg_mask,
                    obs_zero_flag):
    # table2[2i]=1-p_i, table2[2i+1]=p_i, plus a zero row at the end
    p = np.asarray(posterior_prob).astype(np.float32)
    t2 = np.empty((NROWS + 1) * P, dtype=ml_dtypes.bfloat16)
    t2[0:2 * N:2] = (1.0 - p).astype(ml_dtypes.bfloat16)
    t2[1:2 * N:2] = p.astype(ml_dtypes.bfloat16)
    t2[2 * N:] = ml_dtypes.bfloat16(0.0)
    tbl = t2.reshape(NROWS + 1, P)

    iota = np.tile(np.arange(P, dtype=np.float32), (P, 1)) \
        .astype(ml_dtypes.bfloat16)

    inds = np.asarray(latent_var_inds).astype(np.int64)
    mask = np.asarray(latent_neg_mask).astype(np.int64)
    idx2_all = 2 * inds + mask                      # [R, G, K]
    # fold flag into plane 0: flag==False -> dead (zero row) -> Z=0
    dead0 = np.where(np.asarray(obs_zero_flag), idx2_all[:, :, 0], DEAD)
    idx2_all[:, :, 0] = dead0

    in_maps = []
    for c in range(NCORES):
        rules = slice(RLOC * c, RLOC * (c + 1))
        m = {"tbl": tbl, "iota": iota}
        for k in range(K):
            idx16, e_plane = _prep_core(idx2_all[rules, :, k])
            m[f"idx{k}"] = idx16
            m[f"e{k}"] = e_plane
        in_maps.append(m)
    return in_maps


def kernel(posterior_prob, observed_rule_cnts, rule_weights,
           latent_var_inds, latent_neg_mask, obs_zero_flag):
    observed_rule_cnts = np.asarray(observed_rule_cnts)
    rule_weights = np.asarray(rule_weights)

    if "nc" not in _CACHE:
        _CACHE["nc"] = _build_program()
    nc = _CACHE["nc"]

    in_maps = prepare_in_maps(posterior_prob, latent_var_inds,
                              latent_neg_mask, obs_zero_flag)

    from concourse.bass_utils import run_bass_kernel_spmd
    res = run_bass_kernel_spmd(nc, in_maps, core_ids=list(range(NCORES)))

    s = np.empty(R, dtype=np.float64)
    for c in range(NCORES):
        part = res.results[c]["y"].astype(np.float64).sum(axis=0)   # [RLOC]
        s[RLOC * c:RLOC * (c + 1)] = part
    scores = np.float64(G) + observed_rule_cnts.astype(np.float64) - s
    out = rule_weights.astype(np.float64) @ scores
    return np.asarray([out], dtype=np.float32)



# revision 9
# speedup vs baseline: 2.4197x; 1.1698x over previous
"""Trainium2 Bass kernel for nn_ConditionalMLN.

Math: the reference reduces exactly (cart.sum(-1) == 1 algebraically) to
    out = sum_r w_r * (G + cnt_r - S_r),   S_r = sum_g flag[r,g] * Z[r,g]
    Z = prod_k t_k,  t_k = select(mask_k, p[i_k], 1 - p[i_k])

Device strategy (R sharded over 8 cores, 2 rules each). Only flagged
groundings contribute to S_r, so the host COMPACTS each rule's flagged
groundings (~100K of 200K) into a fixed padded slot space — halving the
gather work vs. gathering all G groundings.

  * host builds a doubled bf16 table: table2[2i+m] = m ? p_i : 1-p_i,
    plus one zero row; pad slots point at the zero row (t=0 -> Z=0).
  * flagged groundings are sorted by plane-0 table row for HBM locality.
  * dma_gather (SWDGE row gather, 256B rows of 128 bf16) fetches the
    row containing each element's entry, striped over 4 SWDGE queues.
  * DVE extracts entry e via onehot(iota==e) multiply + windowed
    tensor_reduce, multiplies the three planes (Z), and reduces each
    tile to a per-partition partial sum.
  * host: S_r = partials of rule r summed; out = sum_r w_r (G+cnt_r-S_r).
"""

import numpy as np
import ml_dtypes

R, G, K, N = 16, 200000, 3, 2000000
NCORES = 8
P = 128
RLOC = R // NCORES              # 2 rules per core

# --- compacted slot space (per rule) -----------------------------------
TCOLS = 32                      # columns per gather tile
CS = 800                        # columns per rule (25 tiles); 102400 slots
CAP = CS * P                    # slot capacity per rule (>= ~100K flagged)
COLS = RLOC * CS                # 1600 columns per core
NSLOT = COLS * P                # 204800 slots per core
NTILE = COLS // TCOLS           # 50 tiles (25 per rule)
TSLOT = TCOLS * P               # 4096 slots per tile

NROWS = 2 * N // P              # 31250 rows of 128 bf16 entries in table2
DEAD = 2 * N                    # flat index of the zero row (row NROWS, e=0)
NQ = 4                          # SWDGE queues

_CACHE = {}


def _build_program():
    from concourse import bass, mybir, library_config

    nc = bass.Bass("TRN2", target_bir_lowering=False, debug=False,
                   num_devices=NCORES, num_swdge_queues=NQ)

    f32, bf16, i16 = mybir.dt.float32, mybir.dt.bfloat16, mybir.dt.int16

    tbl_d = nc.declare_dram_parameter("tbl", [NROWS + 1, P], bf16,
                                      isOutput=False)
    idx_d = [nc.declare_dram_parameter(f"idx{k}", [P, NSLOT // 16], i16,
                                       isOutput=False) for k in range(K)]
    e_d = [nc.declare_dram_parameter(f"e{k}", [P, COLS], bf16,
                                     isOutput=False) for k in range(K)]
    iota_d = nc.declare_dram_parameter("iota", [P, P], bf16, isOutput=False)
    y_d = nc.declare_dram_parameter("y", [P, NTILE], f32, isOutput=True)

    idx_s = [[nc.alloc_sbuf_tensor(f"idx{k}_{b}", [P, TCOLS * 8], i16)
              for b in range(NQ)] for k in range(K)]
    rows_s = [[nc.alloc_sbuf_tensor(f"rows{k}_{b}", [P, TSLOT], bf16)
               for b in range(NQ)] for k in range(K)]
    e_s = [nc.alloc_sbuf_tensor(f"e{k}_s", [P, COLS], bf16) for k in range(K)]
    iota_s = nc.alloc_sbuf_tensor("iota_s", [P, P], bf16)
    oh_s = nc.alloc_sbuf_tensor("oh_s", [P, TSLOT], bf16)
    ext_s = [nc.alloc_sbuf_tensor(f"ext{k}_s", [P, TCOLS], f32)
             for k in range(K)]
    part_s = nc.alloc_sbuf_tensor("part_s", [P, NTILE], f32)

    AluOp = mybir.AluOpType
    X = mybir.AxisListType.X

    with (
        nc.Block() as block,
        nc.semaphore("dsem") as dsem,     # e planes + iota loaded
        nc.semaphore("isem") as isem,     # idx tile loads
        nc.semaphore("gsem0") as gsem0,   # gather completions per queue
        nc.semaphore("gsem1") as gsem1,
        nc.semaphore("gsem2") as gsem2,
        nc.semaphore("gsem3") as gsem3,
        nc.semaphore("vsem") as vsem,     # vector tile completions
        nc.semaphore("osem") as osem,
    ):
        gsems = [gsem0, gsem1, gsem2, gsem3]

        @block.sync
        def _(sync):
            for k in range(K):
                sync.dma_start(out=e_s[k].ap(), in_=e_d[k][:]).then_inc(dsem, 16)
            sync.dma_start(out=iota_s.ap(), in_=iota_d[:]).then_inc(dsem, 16)
            for t in range(NTILE):
                if t >= NQ:
                    # idx buf t%NQ free once gathers of tile t-NQ completed
                    sync.wait_ge(gsems[t % NQ], 16 * K * (t // NQ))
                for k in range(K):
                    sync.dma_start(
                        out=idx_s[k][t % NQ].ap(),
                        in_=idx_d[k][:, t * TCOLS * 8:(t + 1) * TCOLS * 8],
                    ).then_inc(isem, 16)
            sync.wait_ge(vsem, NTILE)
            sync.dma_start(out=y_d[:], in_=part_s.ap()).then_inc(osem, 16)
            sync.wait_ge(osem, 16)

        @block.gpsimd
        def _(g):
            g.load_library(library_config.mlp)
            ns_reg = g.to_reg(TSLOT)
            for t in range(NTILE):
                g.wait_ge(isem, 16 * K * (t + 1))
                if t >= NQ:
                    g.wait_ge(vsem, t - NQ + 1)   # rows buf free
                for k in range(K):
                    out3 = rows_s[k][t % NQ].ap().rearrange(
                        "p (c l) -> p c l", l=P)
                    g.dma_gather(
                        out_ap=out3,
                        in_ap=tbl_d[:],
                        idxs_ap=idx_s[k][t % NQ].ap(),
                        num_idxs=TSLOT,
                        num_idxs_reg=ns_reg,
                        elem_size=P,
                        single_packet=False,
                        queue_num=t % NQ,
                    ).then_inc(gsems[t % NQ], 16)

        @block.vector
        def _(v):
            v.wait_ge(dsem, 16 * (K + 1))
            for t in range(NTILE):
                c0 = t * TCOLS
                v.wait_ge(gsems[t % NQ], 16 * K * (t // NQ + 1))
                iota_b = iota_s.ap().unsqueeze(1).broadcast_to([P, TCOLS, P])
                oh3 = oh_s.ap().rearrange("p (c l) -> p c l", l=P)
                for k in range(K):
                    e_b = e_s[k].ap()[:, c0:c0 + TCOLS].unsqueeze(2) \
                        .broadcast_to([P, TCOLS, P])
                    rows3 = rows_s[k][t % NQ].ap().rearrange(
                        "p (c l) -> p c l", l=P)
                    v.tensor_tensor(out=oh3, in0=iota_b, in1=e_b,
                                    op=AluOp.is_equal)
                    v.tensor_tensor(out=rows3, in0=rows3, in1=oh3,
                                    op=AluOp.mult)
                    v.tensor_reduce(ext_s[k].ap(), rows3, X, AluOp.add)
                # Z = ext0*ext1*ext2 (into ext0), then tile partial sum
                z = ext_s[0].ap()
                v.tensor_tensor(out=z, in0=z, in1=ext_s[1].ap(), op=AluOp.mult)
                v.tensor_tensor(out=z, in0=z, in1=ext_s[2].ap(), op=AluOp.mult)
                v.tensor_reduce(part_s.ap()[:, t:t + 1], z, X,
                                AluOp.add).then_inc(vsem, 1)

    from concourse.library_overlay import lower_extended_insts
    lower_extended_insts(nc)
    return nc


def _prep_core(idx2):
    """idx2: [RLOC, CAP, K] int64 flat table2 indices (mask folded, padded,
    row-sorted per rule).

    Returns (idx16 list per plane [P, NSLOT//16] i16, e_planes [P, COLS] bf16
    per plane). Slot j (within rule r) = r*CAP + j; slot s -> [partition
    s%128, col s//128] to match dma_gather output layout.
    """
    flat = idx2.reshape(RLOC * CAP, K)          # slot-ordered [NSLOT, K]
    idx16s, e_planes = [], []
    for k in range(K):
        col = flat[:, k]
        row = (col >> 7).astype(np.int16)
        e = (col & 127).astype(np.float32).astype(ml_dtypes.bfloat16)
        # e-plane: slot s -> [partition s%128, col s//128]
        e_planes.append(np.ascontiguousarray(e.reshape(COLS, P).T))
        # idx16: wrapped-16 layout replicated across the 8 q7 core groups
        w = row.reshape(NSLOT // 16, 16).T      # [16, NSLOT//16]
        idx16s.append(np.ascontiguousarray(np.tile(w, (8, 1))))
    return idx16s, e_planes


def prepare_in_maps(posterior_prob, latent_var_inds, latent_neg_mask,
                    obs_zero_flag):
    # table2[2i]=1-p_i, table2[2i+1]=p_i, plus a zero row at the end
    p = np.asarray(posterior_prob).astype(np.float32)
    t2 = np.empty((NROWS + 1) * P, dtype=ml_dtypes.bfloat16)
    t2[0:2 * N:2] = (1.0 - p).astype(ml_dtypes.bfloat16)
    t2[1:2 * N:2] = p.astype(ml_dtypes.bfloat16)
    t2[2 * N:] = ml_dtypes.bfloat16(0.0)
    tbl = t2.reshape(NROWS + 1, P)

    iota = np.tile(np.arange(P, dtype=np.float32), (P, 1)) \
        .astype(ml_dtypes.bfloat16)

    inds = np.asarray(latent_var_inds).astype(np.int64)
    mask = np.asarray(latent_neg_mask).astype(np.int64)
    flag = np.asarray(obs_zero_flag)
    idx2_all = 2 * inds + mask                  # [R, G, K]

    in_maps = []
    for c in range(NCORES):
        idx2 = np.full((RLOC, CAP, K), DEAD, dtype=np.int64)
        for rl in range(RLOC):
            r = RLOC * c + rl
            sel = np.flatnonzero(flag[r])       # flagged groundings
            nf = sel.size
            assert nf <= CAP, f"rule {r}: {nf} flagged > capacity {CAP}"
            rows = idx2_all[r, sel, :]          # [nf, K]
            order = np.argsort(rows[:, 0], kind="stable")  # HBM row locality
            idx2[rl, :nf, :] = rows[order]
        idx16s, e_planes = _prep_core(idx2)
        m = {"tbl": tbl, "iota": iota}
        for k in range(K):
            m[f"idx{k}"] = idx16s[k]
            m[f"e{k}"] = e_planes[k]
        in_maps.append(m)
    return in_maps


def kernel(posterior_prob, observed_rule_cnts, rule_weights,
           latent_var_inds, latent_neg_mask, obs_zero_flag):
    observed_rule_cnts = np.asarray(observed_rule_cnts)
    rule_weights = np.asarray(rule_weights)

    if "nc" not in _CACHE:
        _CACHE["nc"] = _build_program()
    nc = _CACHE["nc"]

    in_maps = prepare_in_maps(posterior_prob, latent_var_inds,
                              latent_neg_mask, obs_zero_flag)

    from concourse.bass_utils import run_bass_kernel_spmd
    res = run_bass_kernel_spmd(nc, in_maps, core_ids=list(range(NCORES)))

    tiles_per_rule = NTILE // RLOC
    s = np.empty(R, dtype=np.float64)
    for c in range(NCORES):
        part = res.results[c]["y"].astype(np.float64)   # [P, NTILE]
        for rl in range(RLOC):
            r = RLOC * c + rl
            s[r] = part[:, rl * tiles_per_rule:(rl + 1) * tiles_per_rule].sum()
    scores = np.float64(G) + observed_rule_cnts.astype(np.float64) - s
    out = rule_weights.astype(np.float64) @ scores
    return np.asarray([out], dtype=np.float32)


# revision 10
# speedup vs baseline: 2.5036x; 1.0347x over previous
"""Trainium2 Bass kernel for nn_ConditionalMLN.

Math: the reference reduces exactly (cart.sum(-1) == 1 algebraically) to
    out = sum_r w_r * (G + cnt_r - S_r),   S_r = sum_g flag[r,g] * Z[r,g]
    Z = prod_k t_k,  t_k = select(mask_k, p[i_k], 1 - p[i_k])

Device strategy (R sharded over 8 cores, 2 rules each). Only flagged
groundings contribute to S_r, so the host COMPACTS each rule's flagged
groundings (~100K of 200K) into a fixed padded slot space — halving the
gather work vs. gathering all G groundings.

  * host builds a doubled bf16 table: table2[2i+m] = m ? p_i : 1-p_i,
    plus one zero row; pad slots point at the zero row (t=0 -> Z=0).
  * flagged groundings are sorted by plane-0 table row for HBM locality.
  * dma_gather (SWDGE row gather, 256B rows of 128 bf16) fetches the
    row containing each element's entry, striped over 4 SWDGE queues.
  * DVE extracts entry e via onehot(iota==e) multiply + windowed
    tensor_reduce, multiplies the three planes (Z), and reduces each
    tile to a per-partition partial sum.
  * host: S_r = partials of rule r summed; out = sum_r w_r (G+cnt_r-S_r).
"""

import numpy as np
import ml_dtypes

R, G, K, N = 16, 200000, 3, 2000000
NCORES = 8
P = 128
RLOC = R // NCORES              # 2 rules per core

# --- compacted slot space (per rule) -----------------------------------
TCOLS = 32                      # columns per gather tile
CS = 800                        # columns per rule (25 tiles); 102400 slots
CAP = CS * P                    # slot capacity per rule (>= ~100K flagged)
COLS = RLOC * CS                # 1600 columns per core
NSLOT = COLS * P                # 204800 slots per core
NTILE = COLS // TCOLS           # 50 tiles (25 per rule)
TSLOT = TCOLS * P               # 4096 slots per tile

NROWS = 2 * N // P              # 31250 rows of 128 bf16 entries in table2
DEAD = 2 * N                    # flat index of the zero row (row NROWS, e=0)
NQ = 4                          # SWDGE queues

_CACHE = {}


def _build_program():
    from concourse import bass, mybir, library_config

    nc = bass.Bass("TRN2", target_bir_lowering=False, debug=False,
                   num_devices=NCORES, num_swdge_queues=NQ)

    f32, bf16, i16 = mybir.dt.float32, mybir.dt.bfloat16, mybir.dt.int16

    tbl_d = nc.declare_dram_parameter("tbl", [NROWS + 1, P], bf16,
                                      isOutput=False)
    idx_d = [nc.declare_dram_parameter(f"idx{k}", [P, NSLOT // 16], i16,
                                       isOutput=False) for k in range(K)]
    e_d = [nc.declare_dram_parameter(f"e{k}", [P, COLS], bf16,
                                     isOutput=False) for k in range(K)]
    iota_d = nc.declare_dram_parameter("iota", [P, P], bf16, isOutput=False)
    y_d = nc.declare_dram_parameter("y", [P, NTILE], f32, isOutput=True)

    idx_s = [[nc.alloc_sbuf_tensor(f"idx{k}_{b}", [P, TCOLS * 8], i16)
              for b in range(NQ)] for k in range(K)]
    rows_s = [[nc.alloc_sbuf_tensor(f"rows{k}_{b}", [P, TSLOT], bf16)
               for b in range(NQ)] for k in range(K)]
    e_s = [nc.alloc_sbuf_tensor(f"e{k}_s", [P, COLS], bf16) for k in range(K)]
    iota_s = nc.alloc_sbuf_tensor("iota_s", [P, P], bf16)
    oh_s = nc.alloc_sbuf_tensor("oh_s", [P, TSLOT], bf16)
    ext_s = [nc.alloc_sbuf_tensor(f"ext{k}_s", [P, TCOLS], f32)
             for k in range(K)]
    part_s = nc.alloc_sbuf_tensor("part_s", [P, NTILE], f32)

    AluOp = mybir.AluOpType
    X = mybir.AxisListType.X

    with (
        nc.Block() as block,
        nc.semaphore("dsem") as dsem,     # e planes + iota loaded
        nc.semaphore("isem") as isem,     # idx tile loads
        nc.semaphore("gsem0") as gsem0,   # gather completions per queue
        nc.semaphore("gsem1") as gsem1,
        nc.semaphore("gsem2") as gsem2,
        nc.semaphore("gsem3") as gsem3,
        nc.semaphore("vsem") as vsem,     # vector tile completions
        nc.semaphore("osem") as osem,
    ):
        gsems = [gsem0, gsem1, gsem2, gsem3]

        @block.sync
        def _(sync):
            for k in range(K):
                sync.dma_start(out=e_s[k].ap(), in_=e_d[k][:]).then_inc(dsem, 16)
            sync.dma_start(out=iota_s.ap(), in_=iota_d[:]).then_inc(dsem, 16)
            for t in range(NTILE):
                if t >= NQ:
                    # idx buf t%NQ free once gathers of tile t-NQ completed
                    sync.wait_ge(gsems[t % NQ], 16 * K * (t // NQ))
                for k in range(K):
                    sync.dma_start(
                        out=idx_s[k][t % NQ].ap(),
                        in_=idx_d[k][:, t * TCOLS * 8:(t + 1) * TCOLS * 8],
                    ).then_inc(isem, 16)
            sync.wait_ge(vsem, NTILE)
            sync.dma_start(out=y_d[:], in_=part_s.ap()).then_inc(osem, 16)
            sync.wait_ge(osem, 16)

        @block.gpsimd
        def _(g):
            g.load_library(library_config.mlp)
            ns_reg = g.to_reg(TSLOT)
            for t in range(NTILE):
                g.wait_ge(isem, 16 * K * (t + 1))
                if t >= NQ:
                    g.wait_ge(vsem, t - NQ + 1)   # rows buf free
                for k in range(K):
                    out3 = rows_s[k][t % NQ].ap().rearrange(
                        "p (c l) -> p c l", l=P)
                    g.dma_gather(
                        out_ap=out3,
                        in_ap=tbl_d[:],
                        idxs_ap=idx_s[k][t % NQ].ap(),
                        num_idxs=TSLOT,
                        num_idxs_reg=ns_reg,
                        elem_size=P,
                        single_packet=False,
                        queue_num=t % NQ,
                    ).then_inc(gsems[t % NQ], 16)

        @block.vector
        def _(v):
            v.wait_ge(dsem, 16 * (K + 1))
            for t in range(NTILE):
                c0 = t * TCOLS
                v.wait_ge(gsems[t % NQ], 16 * K * (t // NQ + 1))
                iota_b = iota_s.ap().unsqueeze(1).broadcast_to([P, TCOLS, P])
                oh3 = oh_s.ap().rearrange("p (c l) -> p c l", l=P)
                for k in range(K):
                    e_b = e_s[k].ap()[:, c0:c0 + TCOLS].unsqueeze(2) \
                        .broadcast_to([P, TCOLS, P])
                    rows3 = rows_s[k][t % NQ].ap().rearrange(
                        "p (c l) -> p c l", l=P)
                    v.tensor_tensor(out=oh3, in0=iota_b, in1=e_b,
                                    op=AluOp.is_equal)
                    v.tensor_tensor(out=rows3, in0=rows3, in1=oh3,
                                    op=AluOp.mult)
                    v.tensor_reduce(ext_s[k].ap(), rows3, X, AluOp.add)
                # Z = ext0*ext1*ext2 (into ext0), then tile partial sum
                z = ext_s[0].ap()
                v.tensor_tensor(out=z, in0=z, in1=ext_s[1].ap(), op=AluOp.mult)
                v.tensor_tensor(out=z, in0=z, in1=ext_s[2].ap(), op=AluOp.mult)
                v.tensor_reduce(part_s.ap()[:, t:t + 1], z, X,
                                AluOp.add).then_inc(vsem, 1)

    from concourse.library_overlay import lower_extended_insts
    lower_extended_insts(nc)
    return nc


def _prep_core(idx2):
    """idx2: [RLOC, CAP, K] int64 flat table2 indices (mask folded, padded,
    row-sorted per rule).

    Returns (idx16 list per plane [P, NSLOT//16] i16, e_planes [P, COLS] bf16
    per plane). Slot j (within rule r) = r*CAP + j; slot s -> [partition
    s%128, col s//128] to match dma_gather output layout.
    """
    flat = idx2.reshape(RLOC * CAP, K)          # slot-ordered [NSLOT, K]
    idx16s, e_planes = [], []
    for k in range(K):
        col = flat[:, k]
        row = (col >> 7).astype(np.int16)
        e = (col & 127).astype(np.float32).astype(ml_dtypes.bfloat16)
        # e-plane: slot s -> [partition s%128, col s//128]
        e_planes.append(np.ascontiguousarray(e.reshape(COLS, P).T))
        # idx16: wrapped-16 layout replicated across the 8 q7 core groups
        w = row.reshape(NSLOT // 16, 16).T      # [16, NSLOT//16]
        idx16s.append(np.ascontiguousarray(np.tile(w, (8, 1))))
    return idx16s, e_planes


def prepare_in_maps(posterior_prob, latent_var_inds, latent_neg_mask,
                    obs_zero_flag):
    # table2[2i]=1-p_i, table2[2i+1]=p_i, plus a zero row at the end
    p = np.asarray(posterior_prob).astype(np.float32)
    t2 = np.empty((NROWS + 1) * P, dtype=ml_dtypes.bfloat16)
    t2[0:2 * N:2] = (1.0 - p).astype(ml_dtypes.bfloat16)
    t2[1:2 * N:2] = p.astype(ml_dtypes.bfloat16)
    t2[2 * N:] = ml_dtypes.bfloat16(0.0)
    tbl = t2.reshape(NROWS + 1, P)

    iota = np.tile(np.arange(P, dtype=np.float32), (P, 1)) \
        .astype(ml_dtypes.bfloat16)

    inds = np.asarray(latent_var_inds).astype(np.int64)
    mask = np.asarray(latent_neg_mask).astype(np.int64)
    flag = np.asarray(obs_zero_flag)
    idx2_all = 2 * inds + mask                  # [R, G, K]

    in_maps = []
    for c in range(NCORES):
        idx2 = np.full((RLOC, CAP, K), DEAD, dtype=np.int64)
        for rl in range(RLOC):
            r = RLOC * c + rl
            sel = np.flatnonzero(flag[r])       # flagged groundings
            nf = sel.size
            assert nf <= CAP, f"rule {r}: {nf} flagged > capacity {CAP}"
            idx2[rl, :nf, :] = idx2_all[r, sel, :]
        idx16s, e_planes = _prep_core(idx2)
        m = {"tbl": tbl, "iota": iota}
        for k in range(K):
            m[f"idx{k}"] = idx16s[k]
            m[f"e{k}"] = e_planes[k]
        in_maps.append(m)
    return in_maps


def kernel(posterior_prob, observed_rule_cnts, rule_weights,
           latent_var_inds, latent_neg_mask, obs_zero_flag):
    observed_rule_cnts = np.asarray(observed_rule_cnts)
    rule_weights = np.asarray(rule_weights)

    if "nc" not in _CACHE:
        _CACHE["nc"] = _build_program()
    nc = _CACHE["nc"]

    in_maps = prepare_in_maps(posterior_prob, latent_var_inds,
                              latent_neg_mask, obs_zero_flag)

    from concourse.bass_utils import run_bass_kernel_spmd
    res = run_bass_kernel_spmd(nc, in_maps, core_ids=list(range(NCORES)))

    tiles_per_rule = NTILE // RLOC
    s = np.empty(R, dtype=np.float64)
    for c in range(NCORES):
        part = res.results[c]["y"].astype(np.float64)   # [P, NTILE]
        for rl in range(RLOC):
            r = RLOC * c + rl
            s[r] = part[:, rl * tiles_per_rule:(rl + 1) * tiles_per_rule].sum()
    scores = np.float64(G) + observed_rule_cnts.astype(np.float64) - s
    out = rule_weights.astype(np.float64) @ scores
    return np.asarray([out], dtype=np.float32)
